# revision 57
# baseline (speedup 1.0000x reference)
"""Trainium2 Bass kernel for the pre-LN multi-head attention block.

Sharding: 8 cores = 4 batches x 2 query-row halves, collective-free. Each core
computes all 16 heads for its 512 query rows, with full-T k/v for its batch
(k/v compute duplicated across the 2 cores of a batch).

Per-core scheme (C=1024 channels, T=1024 rows, TQ=512 query rows):
  - x^T [C, T] is loaded directly (host transposes; query rows rotated first)
    and normalized in place to z. LN stats come from ones-vector matmuls
    (partition-dim reduction on the PE); rstd = exp(-0.5*ln(var+eps)) on ACT;
    mean/rstd rows are broadcast across partitions via a DRAM bounce.
  - q^T [C, TQ], k^T [C, T] = W^T z^T keep channels on partitions, so the
    qk-LN gains/biases are per-partition scalars (tensor_scalar); their LN
    stats are again ones-matmuls.
  - v [T, C] natural = z^T.T Wv, stored head-interleaved with a ones column
    every 65 cols (softmax denominator augmentation).
  - scores^T per head = matmul(lhsT=k-hat slice, rhs=q-hat slice); two heads
    per 128-channel chunk via row strips 0-63 / 64-127. exp on ACT with the
    1/8 softmax scale folded in; no max-subtraction (scores are O(1) after
    qk-LN of activations drawn from the reference distribution).
  - attn@v: even head = matmul(lhsT=v_aug [128,65]) at psum base 0 (row 64
    accumulates the denominator); odd head = matmul(lhsT=v [128,64]) writing
    at psum base 64 directly, denominator via a separate ones-matmul tile.
    All PSUM reads in this phase are on ACT (ScalarE and VectorE must not
    touch the same PSUM bank in parallel).
  - denominators are DMA-collected to DRAM, reciprocal'd as one [16, TQ]
    batch, broadcast back per chunk, and multiplied into out^T.
  - proj: y^T = Wp^T out^T + bias -> host transposes/scatters.
"""

from contextlib import ExitStack

import numpy as np

import concourse.bacc as bacc
import concourse.mybir as mybir
import concourse.tile as tile
from concourse.bass_utils import run_bass_kernel_spmd

F32 = mybir.dt.float32
F32R = mybir.dt.float32r
BF16 = mybir.dt.bfloat16
AF = mybir.ActivationFunctionType
OP = mybir.AluOpType

B, T, C = 4, 1024, 1024
H, D = 16, 64
TQ = 512           # query rows per core
NCH = 8            # 128-row chunks of C (or T)
EPS = 1e-5

_CACHE = {}


def _build(stop="full"):
    nc = bacc.Bacc(None, target_bir_lowering=False, debug=False)

    xT_d = nc.declare_dram_parameter("xT", [C, T], F32, isOutput=False)
    wq_d = nc.declare_dram_parameter("wq", [C, C], F32, isOutput=False)
    wk_d = nc.declare_dram_parameter("wk", [C, C], F32, isOutput=False)
    wv_d = nc.declare_dram_parameter("wv", [C, C], F32, isOutput=False)
    wp_d = nc.declare_dram_parameter("wp", [C, C], F32, isOutput=False)
    bq_d = nc.declare_dram_parameter("bq", [C], F32, isOutput=False)
    bk_d = nc.declare_dram_parameter("bk", [C], F32, isOutput=False)
    bv_d = nc.declare_dram_parameter("bv", [C], F32, isOutput=False)
    bp_d = nc.declare_dram_parameter("bp", [C], F32, isOutput=False)
    qg_d = nc.declare_dram_parameter("qg", [C], F32, isOutput=False)
    qb_d = nc.declare_dram_parameter("qb", [C], F32, isOutput=False)
    kg_d = nc.declare_dram_parameter("kg", [C], F32, isOutput=False)
    kb_d = nc.declare_dram_parameter("kb", [C], F32, isOutput=False)
    yT_d = nc.declare_dram_parameter("yT", [C, TQ], F32, isOutput=True)

    with tile.TileContext(nc) as tc, ExitStack() as ctx:
        pool = tc.tile_pool

        def mmr(out, lhsT, rhs, **kw):
            # float32r: single-pass reduced-precision fp32 matmul (1 cyc/row
            # at free dim >= 256, vs 4 for fp32)
            nc.tensor.matmul(out, lhsT.bitcast(F32R), rhs.bitcast(F32R), **kw)
        const = ctx.enter_context(pool(name="const", bufs=1))
        qsbp = ctx.enter_context(pool(name="qsb", bufs=1))
        ksbp = ctx.enter_context(pool(name="ksb", bufs=1))
        vsbp = ctx.enter_context(pool(name="vsb", bufs=1))
        bcp = ctx.enter_context(pool(name="bc", bufs=2))
        rows1 = ctx.enter_context(pool(name="rows1", bufs=1))
        rows2 = ctx.enter_context(pool(name="rows2", bufs=2))
        sqp = ctx.enter_context(pool(name="sq", bufs=2))
        dram = ctx.enter_context(pool(name="dram", bufs=1, space="DRAM"))

        # ---- constants ----
        def vec8(name, d):
            t = const.tile([128, 8], F32, tag=name)
            nc.sync.dma_start(out=t, in_=d.ap().rearrange("(j p) -> p j", p=128))
            return t

        bq8 = vec8("bq8", bq_d)
        bk8 = vec8("bk8", bk_d)
        bp8 = vec8("bp8", bp_d)
        qg8 = vec8("qg8", qg_d)
        qb8 = vec8("qb8", qb_d)
        kg8 = vec8("kg8", kg_d)
        kb8 = vec8("kb8", kb_d)
        ones_blk = const.tile([128, 128], F32, tag="onesblk")
        nc.vector.memset(ones_blk, 1.0)
        ones1 = const.tile([128, 1], F32)
        nc.vector.tensor_copy(out=ones1.bitcast(F32R), in_=ones_blk[:, 0:1])
        eps1 = const.tile([1, 1], F32)
        nc.vector.memset(eps1, EPS)
        bvb = const.tile([128, C], F32)
        nc.sync.dma_start(out=bvb, in_=bv_d.ap().rearrange("c -> () c").to_broadcast([128, C]))

        # persistent activations
        q_sb = qsbp.tile([128, NCH, TQ], F32)      # q^T, later q-hat
        k_sb = ksbp.tile([128, NCH, T], F32)       # k^T, later k-hat
        v_sb = vsbp.tile([128, NCH, H * 65], BF16)  # v head-interleaved + ones col

        v_ones_view = v_sb.rearrange("p i (h x) -> p i h x", x=65)[:, :, :, 64:65]
        nc.vector.tensor_copy(out=v_ones_view,
                              in_=ones_blk.rearrange("p (i h x) -> p i h x", i=NCH, h=H))

        # DRAM scratch
        scr_x = dram.tile([1, 2 * T], F32)
        scr_q = dram.tile([1, 2 * TQ], F32)
        scr_k = dram.tile([1, 2 * T], F32)

        def ln_rows(pack, srow, n, scr):
            """pack[:, 0:n] = mean, pack[:, n:2n] = rstd from raw [sum|sumsq]
            rows in srow; DMA pack to DRAM scratch scr."""
            mu = pack[:, 0:n]
            rs = pack[:, n:2 * n]
            nc.vector.tensor_scalar(out=mu, in0=srow[:, 0:n], scalar1=1.0 / C, scalar2=None, op0=OP.mult)
            ex2 = rows2.tile([1, T], F32, tag="rowtmp")
            nc.vector.tensor_scalar(out=ex2[:, 0:n], in0=srow[:, n:2 * n], scalar1=1.0 / C, scalar2=None, op0=OP.mult)
            musq = rows2.tile([1, T], F32, tag="rowtmp")
            nc.vector.tensor_tensor(out=musq[:, 0:n], in0=mu, in1=mu, op=OP.mult)
            nc.vector.tensor_tensor(out=ex2[:, 0:n], in0=ex2[:, 0:n], in1=musq[:, 0:n], op=OP.subtract)
            nc.scalar.activation(out=ex2[:, 0:n], in_=ex2[:, 0:n], func=AF.Ln, bias=eps1, scale=1.0)
            nc.scalar.activation(out=rs, in_=ex2[:, 0:n], func=AF.Exp, scale=-0.5)
            nc.sync.dma_start(out=scr[:, :], in_=pack)

        # ============ big-load FIFO: x chunks, then wq, then wk (ACT hwdge
        # queue, pre-triggered so the stream starts at t=0 in this order) ====
        xz_ctx = ExitStack()
        xzp = xz_ctx.enter_context(pool(name="xz", bufs=1))
        xts = []
        for j in range(NCH):
            t = xzp.tile([128, T], F32, tag=f"x{j}")
            nc.scalar.dma_start(out=t.bitcast(F32R),
                                in_=xT_d[j * 128:(j + 1) * 128, :].bitcast(F32R))
            xts.append(t)

        wqk_ctx = ExitStack()
        wqp = wqk_ctx.enter_context(pool(name="wqp", bufs=1))
        wkp = wqk_ctx.enter_context(pool(name="wkp", bufs=1))

        def wslab(p, wd, m, tag):
            t = p.tile([128, NCH, 128], F32, tag=tag)
            nc.scalar.dma_start(out=t.bitcast(F32R),
                                in_=wd.ap().rearrange("(j p) c -> p j c", p=128)[:, :, m * 128:(m + 1) * 128].bitcast(F32R))
            return t

        wq_tiles = [wslab(wqp, wq_d, m, f"wq{m}") for m in range(NCH)]
        # wk is a 4-slab ring: slabs 4..7 reuse the buffers of 0..3 (triggered
        # inside the k loop once the earlier slab has been consumed)
        wk_tiles = {m: wslab(wkp, wk_d, m, f"wk{m}") for m in range(4)}

        # ================= phase A: stats, normalize =================
        psA_ctx = ExitStack()
        psA = psA_ctx.enter_context(pool(name="psA", bufs=1, space="PSUM"))
        xstat_ps = psA.tile([1, 2 * T], F32)
        for j in range(NCH):
            sqt = sqp.tile([128, T], F32, tag="sq")
            nc.vector.tensor_tensor(out=sqt.bitcast(F32R), in0=xts[j], in1=xts[j], op=OP.mult)
            st, sp = j == 0, j == NCH - 1
            for n in range(2):
                mmr(xstat_ps[0:1, n * 512:(n + 1) * 512], ones1,
                                 xts[j][:, n * 512:(n + 1) * 512], start=st, stop=sp)
                mmr(xstat_ps[0:1, T + n * 512:T + (n + 1) * 512], ones1,
                                 sqt[:, n * 512:(n + 1) * 512], start=st, stop=sp)
        srow = rows1.tile([1, 2 * T], F32, tag="srow")
        nc.vector.tensor_copy(out=srow, in_=xstat_ps)
        psA_ctx.close()

        xpack = rows1.tile([1, 2 * T], F32, tag="pack")
        ln_rows(xpack, srow, T, scr_x)
        mub = bcp.tile([128, T], F32, tag="bc")
        nc.sync.dma_start(out=mub, in_=scr_x[0:1, 0:T].to_broadcast([128, T]))
        rsb = bcp.tile([128, T], F32, tag="bc")
        nc.sync.dma_start(out=rsb, in_=scr_x[0:1, T:2 * T].to_broadcast([128, T]))
        for j in range(NCH):
            tz = sqp.tile([128, T], F32, tag="sq")
            nc.vector.tensor_tensor(out=tz, in0=xts[j], in1=mub, op=OP.subtract)
            nc.vector.tensor_tensor(out=xts[j].bitcast(F32R), in0=tz, in1=rsb, op=OP.mult)

        # ================= phase B: q / k / v matmuls =================
        mm_ctx = ExitStack()
        mmp = mm_ctx.enter_context(pool(name="mm", bufs=2, space="PSUM"))
        wvp = mm_ctx.enter_context(pool(name="wvpool", bufs=2))

        wv_tiles = {}

        def trig_wv(g):
            t = wvp.tile([128, NCH, 256], F32, tag="wv")
            nc.scalar.dma_start(
                out=t.bitcast(F32R),
                in_=wv_d.ap().rearrange("(j p) c -> p j c", p=128)[:, :, g * 256:(g + 1) * 256].bitcast(F32R))
            wv_tiles[g] = t

        # --- q ---
        for m in range(NCH):
            wsl = wq_tiles[m]
            q_ps = mmp.tile([128, T], F32, tag="mm")
            for j in range(NCH):
                mmr(q_ps[:, 0:TQ], wsl[:, j, :], xts[j][:, 0:TQ],
                                 start=(j == 0), stop=(j == NCH - 1))
            nc.scalar.activation(out=q_sb[:, m, :].bitcast(F32R), in_=q_ps[:, 0:TQ], func=AF.Identity,
                                 bias=bq8[:, m:m + 1], scale=1.0)
        qs_ctx = ExitStack()
        qstatp = qs_ctx.enter_context(pool(name="qstat", bufs=1, space="PSUM"))
        qstat_ps = qstatp.tile([1, 2 * TQ], F32)
        for m in range(NCH):
            sqt = sqp.tile([128, T], F32, tag="sq")
            nc.vector.tensor_tensor(out=sqt[:, 0:TQ].bitcast(F32R), in0=q_sb[:, m, :], in1=q_sb[:, m, :], op=OP.mult)
            mmr(qstat_ps[0:1, 0:TQ], ones1, q_sb[:, m, :],
                             start=(m == 0), stop=(m == NCH - 1))
            mmr(qstat_ps[0:1, TQ:2 * TQ], ones1, sqt[:, 0:TQ],
                             start=(m == 0), stop=(m == NCH - 1))
        qsrow = rows1.tile([1, 2 * T], F32, tag="srow")
        nc.vector.tensor_copy(out=qsrow[:, 0:2 * TQ], in_=qstat_ps)
        qs_ctx.close()
        qpack = rows1.tile([1, 2 * T], F32, tag="pack")
        ln_rows(qpack[:, 0:2 * TQ], qsrow, TQ, scr_q)
        muqb = bcp.tile([128, T], F32, tag="bc")
        nc.sync.dma_start(out=muqb[:, 0:TQ], in_=scr_q[0:1, 0:TQ].to_broadcast([128, TQ]))
        rsqb = bcp.tile([128, T], F32, tag="bc")
        nc.sync.dma_start(out=rsqb[:, 0:TQ], in_=scr_q[0:1, TQ:2 * TQ].to_broadcast([128, TQ]))
        for m in range(NCH):
            t1 = sqp.tile([128, T], F32, tag="sq")
            nc.vector.tensor_tensor(out=t1[:, 0:TQ], in0=q_sb[:, m, :], in1=muqb[:, 0:TQ], op=OP.subtract)
            t2 = sqp.tile([128, T], F32, tag="sq")
            nc.vector.tensor_tensor(out=t2[:, 0:TQ], in0=t1[:, 0:TQ], in1=rsqb[:, 0:TQ], op=OP.mult)
            nc.scalar.activation(out=q_sb[:, m, :].bitcast(F32R), in_=t2[:, 0:TQ],
                                 func=AF.Identity, bias=qb8[:, m:m + 1],
                                 scale=qg8[:, m:m + 1])

        # --- k ---
        for m in range(NCH):
            if m + 4 < NCH:
                wk_tiles[m + 4] = wslab(wkp, wk_d, m + 4, f"wk{m}")
            wsl = wk_tiles[m]
            k_ps = mmp.tile([128, T], F32, tag="mm")
            for n in range(2):
                for j in range(NCH):
                    mmr(k_ps[:, n * 512:(n + 1) * 512], wsl[:, j, :],
                                     xts[j][:, n * 512:(n + 1) * 512],
                                     start=(j == 0), stop=(j == NCH - 1))
            nc.scalar.activation(out=k_sb[:, m, :].bitcast(F32R), in_=k_ps, func=AF.Identity,
                                 bias=bk8[:, m:m + 1], scale=1.0)

        # v weight slabs queue behind the wk ring in the hwdge FIFO
        trig_wv(0)
        trig_wv(1)
        ks_ctx = ExitStack()
        kstatp = ks_ctx.enter_context(pool(name="kstat", bufs=1, space="PSUM"))
        kstat_ps = kstatp.tile([1, 2 * T], F32)
        for m in range(NCH):
            sqt = sqp.tile([128, T], F32, tag="sq")
            nc.vector.tensor_tensor(out=sqt.bitcast(F32R), in0=k_sb[:, m, :], in1=k_sb[:, m, :], op=OP.mult)
            for n in range(2):
                mmr(kstat_ps[0:1, n * 512:(n + 1) * 512], ones1,
                                 k_sb[:, m, n * 512:(n + 1) * 512],
                                 start=(m == 0), stop=(m == NCH - 1))
                mmr(kstat_ps[0:1, T + n * 512:T + (n + 1) * 512], ones1,
                                 sqt[:, n * 512:(n + 1) * 512],
                                 start=(m == 0), stop=(m == NCH - 1))
        ksrow = rows1.tile([1, 2 * T], F32, tag="srow")
        nc.vector.tensor_copy(out=ksrow, in_=kstat_ps)
        ks_ctx.close()
        kpack = rows1.tile([1, 2 * T], F32, tag="pack")
        ln_rows(kpack, ksrow, T, scr_k)
        mukb = bcp.tile([128, T], F32, tag="bc")
        nc.sync.dma_start(out=mukb, in_=scr_k[0:1, 0:T].to_broadcast([128, T]))
        rskb = bcp.tile([128, T], F32, tag="bc")
        nc.sync.dma_start(out=rskb, in_=scr_k[0:1, T:2 * T].to_broadcast([128, T]))
        for m in range(NCH):
            t1 = sqp.tile([128, T], F32, tag="sq")
            nc.vector.tensor_tensor(out=t1, in0=k_sb[:, m, :], in1=mukb, op=OP.subtract)
            t2 = sqp.tile([128, T], F32, tag="sq")
            nc.vector.tensor_tensor(out=t2, in0=t1, in1=rskb, op=OP.mult)
            nc.scalar.activation(out=k_sb[:, m, :].bitcast(F32R), in_=t2,
                                 func=AF.Identity, bias=kb8[:, m:m + 1],
                                 scale=kg8[:, m:m + 1])

        # --- v ---
        for g in range(4):
            if g + 2 < 4:
                trig_wv(g + 2)
            wvsl = wv_tiles[g]
            for i in range(NCH):
                v_ps = mmp.tile([128, T], F32, tag="mm")
                for j in range(NCH):
                    mmr(v_ps[:, 0:256], xts[j][:, i * 128:(i + 1) * 128],
                                     wvsl[:, j, :], start=(j == 0), stop=(j == NCH - 1))
                vout = v_sb.rearrange("p i (h x) -> p i h x", x=65)[:, i, g * 4:(g + 1) * 4, 0:64]
                vin = v_ps[:, 0:256].rearrange("p (h x) -> p h x", x=64)
                nc.vector.tensor_tensor(
                    out=vout, in0=vin,
                    in1=bvb[:, g * 256:(g + 1) * 256].rearrange("p (h x) -> p h x", x=64),
                    op=OP.add)

        mm_ctx.close()
        wqk_ctx.close()
        xz_ctx.close()

        # ================= phase C: attention =================
        osbp = ctx.enter_context(pool(name="osb", bufs=1))
        outT_sb = osbp.tile([128, NCH, TQ], F32)
        wpp = ctx.enter_context(pool(name="wpp", bufs=1))
        youtp = ctx.enter_context(pool(name="yout", bufs=2))
        att_ctx = ExitStack()
        pexpp = att_ctx.enter_context(pool(name="pexp", bufs=6))
        denp = att_ctx.enter_context(pool(name="den", bufs=3))
        rcbp = att_ctx.enter_context(pool(name="rcb", bufs=2))
        scp = att_ctx.enter_context(pool(name="sc", bufs=2, space="PSUM"))
        avpp0 = att_ctx.enter_context(pool(name="avp0", bufs=2, space="PSUM"))
        avpp1 = att_ctx.enter_context(pool(name="avp1", bufs=2, space="PSUM"))

        # prefetch the whole proj weight during attention (x slab is freed)
        wp_sb = wpp.tile([128, NCH, C], F32)
        for j in range(NCH):
            nc.scalar.dma_start(
                out=wp_sb[:, j, :].bitcast(F32R),
                in_=wp_d.ap().rearrange("(j p) c -> p j c", p=128)[:, j, :].bitcast(F32R))



        p_tiles = {}

        def emit_scores(m):
            p_list = []
            for i in range(NCH):
                sc_ps = scp.tile([128, 1024], F32, tag="sc")
                mmr(sc_ps[:, 0:512], k_sb[0:64, m, i * 128:(i + 1) * 128],
                                 q_sb[0:64, m, :], start=True, stop=True)
                mmr(sc_ps[:, 512:1024], k_sb[64:128, m, i * 128:(i + 1) * 128],
                                 q_sb[64:128, m, :], start=True, stop=True)
                p_sb = pexpp.tile([128, 1024], BF16, tag="p")
                nc.scalar.activation(out=p_sb, in_=sc_ps[:, 0:1024], func=AF.Exp, scale=0.125)
                p_list.append(p_sb)
            p_tiles[m] = p_list

        def emit_av(m):
            p_list = p_tiles.pop(m)
            h0, h1 = 2 * m, 2 * m + 1
            av0 = avpp0.tile([65, TQ], F32, tag="av0")
            av1 = avpp1.tile([65, TQ], F32, tag="av1")
            for i in range(NCH):
                st, sp = i == 0, i == NCH - 1
                nc.tensor.matmul(av0, v_sb[:, i, h0 * 65:h0 * 65 + 65],
                                 p_list[i][:, 0:512], start=st, stop=sp)
                nc.tensor.matmul(av1, v_sb[:, i, h1 * 65:h1 * 65 + 65],
                                 p_list[i][:, 512:1024], start=st, stop=sp)
            # denominators: drain PSUM row 64 to SBUF (custom-DVE ops cannot
            # read PSUM), fast approx reciprocal, then one GpSimd
            # partition-broadcast into a base-0 [64, 2*TQ] tile (the ucode
            # ignores a nonzero out partition base)
            dd = denp.tile([1, 2 * TQ], F32, tag="den")
            nc.vector.tensor_copy(out=dd[:, 0:TQ], in_=av0[64:65, :])
            nc.vector.tensor_copy(out=dd[:, TQ:2 * TQ], in_=av1[64:65, :])
            rt = denp.tile([1, 2 * TQ], F32, tag="rect")
            nc.vector.reciprocal_approx_fast(out=rt, in_=dd)
            rbb = rcbp.tile([64, 2 * TQ], F32, tag="rbb")
            nc.gpsimd.partition_broadcast(rbb, rt)
            # drain av values on DVE fused with the 1/den scaling (ACT stays
            # pure-exp during attention; banks are disjoint from score banks)
            nc.vector.tensor_tensor(out=outT_sb[0:64, m, :].bitcast(F32R),
                                    in0=av0[0:64, :], in1=rbb[:, 0:TQ], op=OP.mult)
            nc.vector.tensor_tensor(out=outT_sb[64:128, m, :].bitcast(F32R),
                                    in0=av1[0:64, :], in1=rbb[:, TQ:2 * TQ], op=OP.mult)

        emit_scores(0)
        for m in range(NCH):
            if m + 1 < NCH:
                emit_scores(m + 1)
            emit_av(m)

        att_ctx.close()

        # ================= phase D: proj =================
        pjp = ctx.enter_context(pool(name="pj", bufs=2, space="PSUM"))
        for m in range(NCH):
            y_ps = pjp.tile([128, TQ], F32, tag="pj")
            for j in range(NCH):
                mmr(y_ps, wp_sb[:, j, m * 128:(m + 1) * 128], outT_sb[:, j, :],
                                 start=(j == 0), stop=(j == NCH - 1))
            y_sb = youtp.tile([128, TQ], F32, tag="y")
            nc.scalar.activation(out=y_sb, in_=y_ps, func=AF.Identity,
                                 bias=bp8[:, m:m + 1], scale=1.0)
            nc.sync.dma_start(out=yT_d[m * 128:(m + 1) * 128, :], in_=y_sb)

    nc.finalize()
    return nc


def _get_nc():
    if "nc" not in _CACHE:
        _CACHE["nc"] = _build()
    return _CACHE["nc"]


def _prep_inputs(x, norm_g, norm_b, qkv_w, qkv_b, qln_g, qln_b, kln_g, kln_b, proj_w, proj_b):
    x = np.asarray(x, dtype=np.float32)
    norm_g = np.asarray(norm_g, dtype=np.float32)
    norm_b = np.asarray(norm_b, dtype=np.float32)
    qkv_w = np.asarray(qkv_w, dtype=np.float32)
    qkv_b = np.asarray(qkv_b, dtype=np.float32)

    wfold = norm_g[:, None] * qkv_w                    # [C, 3C]
    bfold = qkv_b + norm_b @ qkv_w                     # [3C]
    wq = np.ascontiguousarray(wfold[:, 0:C])
    wk = np.ascontiguousarray(wfold[:, C:2 * C])
    wv = np.ascontiguousarray(wfold[:, 2 * C:3 * C])
    bq, bk, bv = bfold[0:C].copy(), bfold[C:2 * C].copy(), bfold[2 * C:3 * C].copy()

    common = dict(
        wq=wq, wk=wk, wv=wv,
        wp=np.ascontiguousarray(np.asarray(proj_w, dtype=np.float32)),
        bq=bq, bk=bk, bv=bv,
        bp=np.asarray(proj_b, dtype=np.float32).copy(),
        qg=np.asarray(qln_g, dtype=np.float32).copy(),
        qb=np.asarray(qln_b, dtype=np.float32).copy(),
        kg=np.asarray(kln_g, dtype=np.float32).copy(),
        kb=np.asarray(kln_b, dtype=np.float32).copy(),
    )
    in_maps = []
    for core in range(8):
        b, half = core // 2, core % 2
        xp = np.concatenate([x[b, TQ * half:], x[b, :TQ * half]], axis=0) if half else x[b]
        xT = np.ascontiguousarray(xp.T)
        in_maps.append(dict(common, xT=xT))
    return in_maps


def kernel(**inputs) -> np.ndarray:
    in_maps = _prep_inputs(**inputs)
    nc = _get_nc()
    res = run_bass_kernel_spmd(nc, in_maps, core_ids=list(range(8)))
    out = np.empty((B, T, C), dtype=np.float32)
    for core in range(8):
        b, half = core // 2, core % 2
        out[b, TQ * half:TQ * half + TQ, :] = res.results[core]["yT"].T
    return out



# revision 68
# speedup vs baseline: 1.0165x; 1.0165x over previous
"""Trainium2 Bass kernel for the pre-LN multi-head attention block.

Sharding: 8 cores = 4 batches x 2 query-row halves, collective-free. Each core
computes all 16 heads for its 512 query rows, with full-T k/v for its batch
(k/v compute duplicated across the 2 cores of a batch).

Per-core scheme (C=1024 channels, T=1024 rows, TQ=512 query rows):
  - all matmuls run as float32r (1 cyc/row) or bf16; PSUM accumulates fp32.
  - big loads (x chunks, wq, wk-ring, wv, wp) ride the ACT hwdge DMA queue as
    one FIFO, pre-triggered so x streams first, then weights, with no gaps.
  - x^T chunks [128, T] are normalized in place (LN stats via ones-matmuls on
    the PE; rstd = exp(-0.5*ln(var+eps)) on ACT; mean/rstd broadcast across
    partitions via a DRAM bounce on the sync queue).
  - q^T / k^T keep channels on partitions; qk-LN gain/bias applied on ACT
    (per-partition scale/bias APs), with stats again from ones-matmuls.
  - v [T, C] is bf16, head-interleaved with a ones column every 65 cols
    (softmax denominator accumulates as row 64 of the av psum).
  - the v matmul loop is interleaved with attention per head-group: v weight
    group g produces heads 4g..4g+3, after which score/exp/av for head pairs
    2g and 2g+1 are emitted. This keeps the ACT queue free of LN-apply work
    by the time the 64 softmax exps (the attention-phase ceiling) start.
  - scores^T per head pair = 2 matmuls (K=64 halves of the chunk); exp on ACT
    (scale=0.125 folded in, no max-subtraction); p stored bf16.
  - attn@v: both heads via 65-col augmented v (bf16), psum pool of 3 banks.
  - denominators: psum row 64 -> SBUF (DVE), reciprocal_approx_fast, GpSimd
    partition_broadcast to [64, 2*TQ], then the PSUM drain of av values fuses
    the 1/den scaling (DVE tensor_tensor, banks disjoint from ACT's).
  - proj: y^T = Wp^T out^T + bias from a prefetched wp slab; double-buffered
    psum; host transposes/scatters.
"""

from contextlib import ExitStack

import numpy as np

import concourse.bacc as bacc
import concourse.mybir as mybir
import concourse.tile as tile
from concourse.bass_utils import run_bass_kernel_spmd

F32 = mybir.dt.float32
F32R = mybir.dt.float32r
BF16 = mybir.dt.bfloat16
AF = mybir.ActivationFunctionType
OP = mybir.AluOpType

B, T, C = 4, 1024, 1024
H, D = 16, 64
TQ = 512           # query rows per core
NCH = 8            # 128-row chunks of C (or T)
EPS = 1e-5

_CACHE = {}


def _build():
    nc = bacc.Bacc(None, target_bir_lowering=False, debug=False)

    xT_d = nc.declare_dram_parameter("xT", [C, T], F32, isOutput=False)
    wq_d = nc.declare_dram_parameter("wq", [C, C], F32, isOutput=False)
    wk_d = nc.declare_dram_parameter("wk", [C, C], F32, isOutput=False)
    wv_d = nc.declare_dram_parameter("wv", [C, C], F32, isOutput=False)
    wp_d = nc.declare_dram_parameter("wp", [C, C], F32, isOutput=False)
    bq_d = nc.declare_dram_parameter("bq", [C], F32, isOutput=False)
    bk_d = nc.declare_dram_parameter("bk", [C], F32, isOutput=False)
    bv_d = nc.declare_dram_parameter("bv", [C], F32, isOutput=False)
    bp_d = nc.declare_dram_parameter("bp", [C], F32, isOutput=False)
    qg_d = nc.declare_dram_parameter("qg", [C], F32, isOutput=False)
    qb_d = nc.declare_dram_parameter("qb", [C], F32, isOutput=False)
    kg_d = nc.declare_dram_parameter("kg", [C], F32, isOutput=False)
    kb_d = nc.declare_dram_parameter("kb", [C], F32, isOutput=False)
    yT_d = nc.declare_dram_parameter("yT", [C, TQ], F32, isOutput=True)

    with tile.TileContext(nc) as tc, ExitStack() as ctx:
        pool = tc.tile_pool

        def mmr(out, lhsT, rhs, **kw):
            # float32r: single-pass reduced-precision fp32 matmul (1 cyc/row
            # at free dim >= 256, vs 4 for fp32)
            nc.tensor.matmul(out, lhsT.bitcast(F32R), rhs.bitcast(F32R), **kw)

        const = ctx.enter_context(pool(name="const", bufs=1))
        qsbp = ctx.enter_context(pool(name="qsb", bufs=1))
        ksbp = ctx.enter_context(pool(name="ksb", bufs=1))
        vsbp = ctx.enter_context(pool(name="vsb", bufs=1))
        dram = ctx.enter_context(pool(name="dram", bufs=1, space="DRAM"))
        xzp = ctx.enter_context(pool(name="xz", bufs=1))
        wvp = ctx.enter_context(pool(name="wvpool", bufs=2))

        # temp pools for the LN-stat phases; closed before attention so the
        # attention pools fit in SBUF (pool frees are LIFO: these sit above
        # the whole-kernel pools and below the wk/wq slab pools)
        tmp_ctx = ExitStack()
        bcp = tmp_ctx.enter_context(pool(name="bc", bufs=2))
        rows1 = tmp_ctx.enter_context(pool(name="rows1", bufs=1))
        rows2 = tmp_ctx.enter_context(pool(name="rows2", bufs=2))
        sqp = tmp_ctx.enter_context(pool(name="sq", bufs=2))

        # ---- constants ----
        def vec8(name, d):
            t = const.tile([128, 8], F32, tag=name)
            nc.sync.dma_start(out=t, in_=d.ap().rearrange("(j p) -> p j", p=128))
            return t

        bq8 = vec8("bq8", bq_d)
        bk8 = vec8("bk8", bk_d)
        bp8 = vec8("bp8", bp_d)
        qg8 = vec8("qg8", qg_d)
        qb8 = vec8("qb8", qb_d)
        kg8 = vec8("kg8", kg_d)
        kb8 = vec8("kb8", kb_d)
        ones_blk = const.tile([128, 128], F32, tag="onesblk")
        nc.vector.memset(ones_blk, 1.0)
        ones1 = const.tile([128, 1], F32)
        nc.vector.tensor_copy(out=ones1.bitcast(F32R), in_=ones_blk[:, 0:1])
        eps1 = const.tile([1, 1], F32)
        nc.vector.memset(eps1, EPS)
        bvb = const.tile([128, C], F32)
        nc.sync.dma_start(out=bvb, in_=bv_d.ap().rearrange("c -> () c").to_broadcast([128, C]))

        # persistent activations
        q_sb = qsbp.tile([128, NCH, TQ], F32)      # q^T, later q-hat
        k_sb = ksbp.tile([128, NCH, T], F32)       # k^T, later k-hat
        v_sb = vsbp.tile([128, NCH, H * 65], BF16)  # v head-interleaved + ones col

        v_ones_view = v_sb.rearrange("p i (h x) -> p i h x", x=65)[:, :, :, 64:65]
        nc.vector.tensor_copy(out=v_ones_view,
                              in_=ones_blk.rearrange("p (i h x) -> p i h x", i=NCH, h=H))

        # DRAM scratch
        scr_x = dram.tile([1, 2 * T], F32)
        scr_q = dram.tile([1, 2 * TQ], F32)
        scr_k = dram.tile([1, 2 * T], F32)

        def ln_rows(pack, srow, n, scr):
            """pack[:, 0:n] = mean, pack[:, n:2n] = rstd from raw [sum|sumsq]
            rows in srow; DMA pack to DRAM scratch scr."""
            mu = pack[:, 0:n]
            rs = pack[:, n:2 * n]
            nc.vector.tensor_scalar(out=mu, in0=srow[:, 0:n], scalar1=1.0 / C, scalar2=None, op0=OP.mult)
            ex2 = rows2.tile([1, T], F32, tag="rowtmp")
            nc.vector.tensor_scalar(out=ex2[:, 0:n], in0=srow[:, n:2 * n], scalar1=1.0 / C, scalar2=None, op0=OP.mult)
            musq = rows2.tile([1, T], F32, tag="rowtmp")
            nc.vector.tensor_tensor(out=musq[:, 0:n], in0=mu, in1=mu, op=OP.mult)
            nc.vector.tensor_tensor(out=ex2[:, 0:n], in0=ex2[:, 0:n], in1=musq[:, 0:n], op=OP.subtract)
            nc.scalar.activation(out=ex2[:, 0:n], in_=ex2[:, 0:n], func=AF.Ln, bias=eps1, scale=1.0)
            nc.scalar.activation(out=rs, in_=ex2[:, 0:n], func=AF.Exp, scale=-0.5)
            nc.sync.dma_start(out=scr[:, :], in_=pack)

        # ============ big-load FIFO: x chunks, then wq, then wk (ACT hwdge
        # queue, pre-triggered so the stream starts at t=0 in this order) ====
        xts = []
        for j in range(NCH):
            t = xzp.tile([128, T], F32, tag=f"x{j}")
            nc.scalar.dma_start(out=t.bitcast(F32R),
                                in_=xT_d[j * 128:(j + 1) * 128, :].bitcast(F32R))
            xts.append(t)

        wk_ctx = ExitStack()
        wkp = wk_ctx.enter_context(pool(name="wkp", bufs=1))
        wq_ctx = ExitStack()
        wqp = wq_ctx.enter_context(pool(name="wqp", bufs=1))

        def wslab(p, wd, m, tag):
            t = p.tile([128, NCH, 128], F32, tag=tag)
            nc.scalar.dma_start(out=t.bitcast(F32R),
                                in_=wd.ap().rearrange("(j p) c -> p j c", p=128)[:, :, m * 128:(m + 1) * 128].bitcast(F32R))
            return t

        wq_tiles = [wslab(wqp, wq_d, m, f"wq{m}") for m in range(NCH)]
        # wk is a 4-slab ring: slabs 4..7 reuse the buffers of 0..3 (triggered
        # inside the k loop once the earlier slab has been consumed)
        wk_tiles = {m: wslab(wkp, wk_d, m, f"wk{m}") for m in range(4)}

        # ================= phase A: stats, normalize =================
        psA_ctx = ExitStack()
        psA = psA_ctx.enter_context(pool(name="psA", bufs=1, space="PSUM"))
        xstat_ps = psA.tile([1, 2 * T], F32)
        for j in range(NCH):
            sqt = sqp.tile([128, T], F32, tag="sq")
            nc.vector.tensor_tensor(out=sqt.bitcast(F32R), in0=xts[j], in1=xts[j], op=OP.mult)
            st, sp = j == 0, j == NCH - 1
            for n in range(2):
                mmr(xstat_ps[0:1, n * 512:(n + 1) * 512], ones1,
                                 xts[j][:, n * 512:(n + 1) * 512], start=st, stop=sp)
                mmr(xstat_ps[0:1, T + n * 512:T + (n + 1) * 512], ones1,
                                 sqt[:, n * 512:(n + 1) * 512], start=st, stop=sp)
        srow = rows1.tile([1, 2 * T], F32, tag="srow")
        nc.vector.tensor_copy(out=srow, in_=xstat_ps)
        psA_ctx.close()

        xpack = rows1.tile([1, 2 * T], F32, tag="pack")
        ln_rows(xpack, srow, T, scr_x)
        mub = bcp.tile([128, T], F32, tag="bc")
        nc.sync.dma_start(out=mub, in_=scr_x[0:1, 0:T].to_broadcast([128, T]))
        rsb = bcp.tile([128, T], F32, tag="bc")
        nc.sync.dma_start(out=rsb, in_=scr_x[0:1, T:2 * T].to_broadcast([128, T]))
        for j in range(NCH):
            tz = sqp.tile([128, T], F32, tag="sq")
            nc.vector.tensor_tensor(out=tz, in0=xts[j], in1=mub, op=OP.subtract)
            nc.vector.tensor_tensor(out=xts[j].bitcast(F32R), in0=tz, in1=rsb, op=OP.mult)

        # ================= phase B: q / k matmuls + their LNs =================
        qmm_ctx = ExitStack()
        qmmp = qmm_ctx.enter_context(pool(name="qmm", bufs=2, space="PSUM"))

        # --- q ---
        for m in range(NCH):
            wsl = wq_tiles[m]
            q_ps = qmmp.tile([128, T], F32, tag="mm")
            for j in range(NCH):
                mmr(q_ps[:, 0:TQ], wsl[:, j, :], xts[j][:, 0:TQ],
                                 start=(j == 0), stop=(j == NCH - 1))
            nc.scalar.activation(out=q_sb[:, m, :].bitcast(F32R), in_=q_ps[:, 0:TQ], func=AF.Identity,
                                 bias=bq8[:, m:m + 1], scale=1.0)
        qmm_ctx.close()
        qs_ctx = ExitStack()
        qstatp = qs_ctx.enter_context(pool(name="qstat", bufs=1, space="PSUM"))
        qstat_ps = qstatp.tile([1, 2 * TQ], F32)
        for m in range(NCH):
            sqt = sqp.tile([128, T], F32, tag="sq")
            nc.vector.tensor_tensor(out=sqt[:, 0:TQ].bitcast(F32R), in0=q_sb[:, m, :], in1=q_sb[:, m, :], op=OP.mult)
            mmr(qstat_ps[0:1, 0:TQ], ones1, q_sb[:, m, :],
                             start=(m == 0), stop=(m == NCH - 1))
            mmr(qstat_ps[0:1, TQ:2 * TQ], ones1, sqt[:, 0:TQ],
                             start=(m == 0), stop=(m == NCH - 1))
        qsrow = rows1.tile([1, 2 * T], F32, tag="srow")
        nc.vector.tensor_copy(out=qsrow[:, 0:2 * TQ], in_=qstat_ps)
        qs_ctx.close()
        qpack = rows1.tile([1, 2 * T], F32, tag="pack")
        ln_rows(qpack[:, 0:2 * TQ], qsrow, TQ, scr_q)
        muqb = bcp.tile([128, T], F32, tag="bc")
        nc.sync.dma_start(out=muqb[:, 0:TQ], in_=scr_q[0:1, 0:TQ].to_broadcast([128, TQ]))
        rsqb = bcp.tile([128, T], F32, tag="bc")
        nc.sync.dma_start(out=rsqb[:, 0:TQ], in_=scr_q[0:1, TQ:2 * TQ].to_broadcast([128, TQ]))
        for m in range(NCH):
            t1 = sqp.tile([128, T], F32, tag="sq")
            nc.vector.tensor_tensor(out=t1[:, 0:TQ], in0=q_sb[:, m, :], in1=muqb[:, 0:TQ], op=OP.subtract)
            t2 = sqp.tile([128, T], F32, tag="sq")
            nc.vector.tensor_tensor(out=t2[:, 0:TQ], in0=t1[:, 0:TQ], in1=rsqb[:, 0:TQ], op=OP.mult)
            nc.scalar.activation(out=q_sb[:, m, :].bitcast(F32R), in_=t2[:, 0:TQ],
                                 func=AF.Identity, bias=qb8[:, m:m + 1],
                                 scale=qg8[:, m:m + 1])
        wq_ctx.close()

        # --- k ---
        kmm_ctx = ExitStack()
        kmmp = kmm_ctx.enter_context(pool(name="kmm", bufs=2, space="PSUM"))
        for m in range(NCH):
            if m + 4 < NCH:
                wk_tiles[m + 4] = wslab(wkp, wk_d, m + 4, f"wk{m}")
            wsl = wk_tiles[m]
            k_ps = kmmp.tile([128, T], F32, tag="mm")
            for n in range(2):
                for j in range(NCH):
                    mmr(k_ps[:, n * 512:(n + 1) * 512], wsl[:, j, :],
                                     xts[j][:, n * 512:(n + 1) * 512],
                                     start=(j == 0), stop=(j == NCH - 1))
            nc.scalar.activation(out=k_sb[:, m, :].bitcast(F32R), in_=k_ps, func=AF.Identity,
                                 bias=bk8[:, m:m + 1], scale=1.0)

        # v weight slabs queue behind the wk ring in the hwdge FIFO
        wv_tiles = {}

        def trig_wv(g):
            t = wvp.tile([128, NCH, 256], F32, tag="wv")
            nc.scalar.dma_start(
                out=t.bitcast(F32R),
                in_=wv_d.ap().rearrange("(j p) c -> p j c", p=128)[:, :, g * 256:(g + 1) * 256].bitcast(F32R))
            wv_tiles[g] = t

        trig_wv(0)
        trig_wv(1)

        ks_ctx = ExitStack()
        kstatp = ks_ctx.enter_context(pool(name="kstat", bufs=1, space="PSUM"))
        kstat_ps = kstatp.tile([1, 2 * T], F32)
        for m in range(NCH):
            sqt = sqp.tile([128, T], F32, tag="sq")
            nc.vector.tensor_tensor(out=sqt.bitcast(F32R), in0=k_sb[:, m, :], in1=k_sb[:, m, :], op=OP.mult)
            for n in range(2):
                mmr(kstat_ps[0:1, n * 512:(n + 1) * 512], ones1,
                                 k_sb[:, m, n * 512:(n + 1) * 512],
                                 start=(m == 0), stop=(m == NCH - 1))
                mmr(kstat_ps[0:1, T + n * 512:T + (n + 1) * 512], ones1,
                                 sqt[:, n * 512:(n + 1) * 512],
                                 start=(m == 0), stop=(m == NCH - 1))
        ksrow = rows1.tile([1, 2 * T], F32, tag="srow")
        nc.vector.tensor_copy(out=ksrow, in_=kstat_ps)
        ks_ctx.close()
        kpack = rows1.tile([1, 2 * T], F32, tag="pack")
        ln_rows(kpack, ksrow, T, scr_k)
        mukb = bcp.tile([128, T], F32, tag="bc")
        nc.sync.dma_start(out=mukb, in_=scr_k[0:1, 0:T].to_broadcast([128, T]))
        rskb = bcp.tile([128, T], F32, tag="bc")
        nc.sync.dma_start(out=rskb, in_=scr_k[0:1, T:2 * T].to_broadcast([128, T]))
        for m in range(NCH):
            t1 = sqp.tile([128, T], F32, tag="sq")
            nc.vector.tensor_tensor(out=t1, in0=k_sb[:, m, :], in1=mukb, op=OP.subtract)
            t2 = sqp.tile([128, T], F32, tag="sq")
            nc.vector.tensor_tensor(out=t2, in0=t1, in1=rskb, op=OP.mult)
            nc.scalar.activation(out=k_sb[:, m, :].bitcast(F32R), in_=t2,
                                 func=AF.Identity, bias=kb8[:, m:m + 1],
                                 scale=kg8[:, m:m + 1])

        kmm_ctx.close()
        wk_ctx.close()
        tmp_ctx.close()

        # ================= phase C: v interleaved with attention =============
        osbp = ctx.enter_context(pool(name="osb", bufs=1))
        outT_sb = osbp.tile([128, NCH, TQ], F32)
        wpp = ctx.enter_context(pool(name="wpp", bufs=1))
        youtp = ctx.enter_context(pool(name="yout", bufs=2))
        att_ctx = ExitStack()
        pexpp = att_ctx.enter_context(pool(name="pexp", bufs=7))
        denp = att_ctx.enter_context(pool(name="den", bufs=2))
        rcbp = att_ctx.enter_context(pool(name="rcb", bufs=2))
        vpsp = att_ctx.enter_context(pool(name="vps", bufs=1, space="PSUM"))
        scp = att_ctx.enter_context(pool(name="sc", bufs=2, space="PSUM"))
        avp = att_ctx.enter_context(pool(name="av", bufs=3, space="PSUM"))

        # prefetch the whole proj weight during attention (wq slab is freed)
        wp_sb = wpp.tile([128, NCH, C], F32)
        for j in range(NCH):
            nc.scalar.dma_start(
                out=wp_sb[:, j, :].bitcast(F32R),
                in_=wp_d.ap().rearrange("(j p) c -> p j c", p=128)[:, j, :].bitcast(F32R))

        p_tiles = {}

        def emit_scores(m):
            p_list = []
            for i in range(NCH):
                sc_ps = scp.tile([128, 1024], F32, tag="sc")
                mmr(sc_ps[:, 0:512], k_sb[0:64, m, i * 128:(i + 1) * 128],
                                 q_sb[0:64, m, :], start=True, stop=True)
                mmr(sc_ps[:, 512:1024], k_sb[64:128, m, i * 128:(i + 1) * 128],
                                 q_sb[64:128, m, :], start=True, stop=True)
                p_sb = pexpp.tile([128, 1024], BF16, tag="p")
                nc.scalar.activation(out=p_sb, in_=sc_ps[:, 0:1024], func=AF.Exp, scale=0.125)
                p_list.append(p_sb)
            p_tiles[m] = p_list

        def emit_av(m):
            p_list = p_tiles.pop(m)
            h0, h1 = 2 * m, 2 * m + 1
            av0 = avp.tile([65, TQ], F32, tag="av")
            av1 = avp.tile([65, TQ], F32, tag="av")
            for i in range(NCH):
                st, sp = i == 0, i == NCH - 1
                nc.tensor.matmul(av0, v_sb[:, i, h0 * 65:h0 * 65 + 65],
                                 p_list[i][:, 0:512], start=st, stop=sp)
                nc.tensor.matmul(av1, v_sb[:, i, h1 * 65:h1 * 65 + 65],
                                 p_list[i][:, 512:1024], start=st, stop=sp)
            # denominators: drain PSUM row 64 to SBUF (custom-DVE ops cannot
            # read PSUM), fast approx reciprocal, then one GpSimd
            # partition-broadcast into a base-0 [64, 2*TQ] tile (the ucode
            # ignores a nonzero out partition base)
            dd = denp.tile([1, 2 * TQ], F32, tag="den")
            nc.vector.tensor_copy(out=dd[:, 0:TQ], in_=av0[64:65, :])
            nc.vector.tensor_copy(out=dd[:, TQ:2 * TQ], in_=av1[64:65, :])
            rt = denp.tile([1, 2 * TQ], F32, tag="rect")
            nc.vector.reciprocal_approx_fast(out=rt, in_=dd)
            rbb = rcbp.tile([64, 2 * TQ], F32, tag="rbb")
            nc.gpsimd.partition_broadcast(rbb, rt)
            # drain av values on DVE fused with the 1/den scaling (ACT stays
            # pure-exp during attention; banks are disjoint from score banks)
            nc.vector.tensor_tensor(out=outT_sb[0:64, m, :].bitcast(F32R),
                                    in0=av0[0:64, :], in1=rbb[:, 0:TQ], op=OP.mult)
            nc.vector.tensor_tensor(out=outT_sb[64:128, m, :].bitcast(F32R),
                                    in0=av1[0:64, :], in1=rbb[:, TQ:2 * TQ], op=OP.mult)

        # --- v group g feeds head pairs 2g and 2g+1 ---
        for g in range(4):
            if g + 2 < 4:
                trig_wv(g + 2)
            wvsl = wv_tiles[g]
            for i in range(NCH):
                v_ps = vpsp.tile([128, 256], F32, tag="vps")
                for j in range(NCH):
                    mmr(v_ps, xts[j][:, i * 128:(i + 1) * 128],
                                     wvsl[:, j, :], start=(j == 0), stop=(j == NCH - 1))
                vout = v_sb.rearrange("p i (h x) -> p i h x", x=65)[:, i, g * 4:(g + 1) * 4, 0:64]
                vin = v_ps.rearrange("p (h x) -> p h x", x=64)
                nc.vector.tensor_tensor(
                    out=vout, in0=vin,
                    in1=bvb[:, g * 256:(g + 1) * 256].rearrange("p (h x) -> p h x", x=64),
                    op=OP.add)
            emit_scores(2 * g)
            emit_av(2 * g)
            emit_scores(2 * g + 1)
            emit_av(2 * g + 1)

        att_ctx.close()

        # ================= phase D: proj =================
        pjp = ctx.enter_context(pool(name="pj", bufs=2, space="PSUM"))
        for m in range(NCH):
            y_ps = pjp.tile([128, TQ], F32, tag="pj")
            for j in range(NCH):
                mmr(y_ps, wp_sb[:, j, m * 128:(m + 1) * 128], outT_sb[:, j, :],
                                 start=(j == 0), stop=(j == NCH - 1))
            y_sb = youtp.tile([128, TQ], F32, tag="y")
            nc.scalar.activation(out=y_sb, in_=y_ps, func=AF.Identity,
                                 bias=bp8[:, m:m + 1], scale=1.0)
            nc.sync.dma_start(out=yT_d[m * 128:(m + 1) * 128, :], in_=y_sb)

    nc.finalize()
    return nc


def _get_nc():
    if "nc" not in _CACHE:
        _CACHE["nc"] = _build()
    return _CACHE["nc"]


def _prep_inputs(x, norm_g, norm_b, qkv_w, qkv_b, qln_g, qln_b, kln_g, kln_b, proj_w, proj_b):
    x = np.asarray(x, dtype=np.float32)
    norm_g = np.asarray(norm_g, dtype=np.float32)
    norm_b = np.asarray(norm_b, dtype=np.float32)
    qkv_w = np.asarray(qkv_w, dtype=np.float32)
    qkv_b = np.asarray(qkv_b, dtype=np.float32)

    wfold = norm_g[:, None] * qkv_w                    # [C, 3C]
    bfold = qkv_b + norm_b @ qkv_w                     # [3C]
    wq = np.ascontiguousarray(wfold[:, 0:C])
    wk = np.ascontiguousarray(wfold[:, C:2 * C])
    wv = np.ascontiguousarray(wfold[:, 2 * C:3 * C])
    bq, bk, bv = bfold[0:C].copy(), bfold[C:2 * C].copy(), bfold[2 * C:3 * C].copy()

    common = dict(
        wq=wq, wk=wk, wv=wv,
        wp=np.ascontiguousarray(np.asarray(proj_w, dtype=np.float32)),
        bq=bq, bk=bk, bv=bv,
        bp=np.asarray(proj_b, dtype=np.float32).copy(),
        qg=np.asarray(qln_g, dtype=np.float32).copy(),
        qb=np.asarray(qln_b, dtype=np.float32).copy(),
        kg=np.asarray(kln_g, dtype=np.float32).copy(),
        kb=np.asarray(kln_b, dtype=np.float32).copy(),
    )
    in_maps = []
    for core in range(8):
        b, half = core // 2, core % 2
        xp = np.concatenate([x[b, TQ * half:], x[b, :TQ * half]], axis=0) if half else x[b]
        xT = np.ascontiguousarray(xp.T)
        in_maps.append(dict(common, xT=xT))
    return in_maps


def kernel(**inputs) -> np.ndarray:
    in_maps = _prep_inputs(**inputs)
    nc = _get_nc()
    res = run_bass_kernel_spmd(nc, in_maps, core_ids=list(range(8)))
    out = np.empty((B, T, C), dtype=np.float32)
    for core in range(8):
        b, half = core // 2, core % 2
        out[b, TQ * half:TQ * half + TQ, :] = res.results[core]["yT"].T
    return out
